# revision 15
# baseline (speedup 1.0000x reference)
"""Trainium2 Bass kernel for nn_AttentionHead (Gaussian mask rasterization).

Reference computation (per batch sample b of 16, per mask n of 50):
    mask[n,i,j] = factor[n] * exp(-0.5*(dx2[n,i] + dy2[n,j]))     [256,256]
    out = (mask - min) / (max - min) * 50         (min/max over all n,i,j of b)
    channel-shuffled on n, labels likewise.

The Gaussian is separable: mask[i,j] = exf[i] * ey[j] with all the
normalization folded into exf on the host (the per-sample min underflows to
exactly 0 in fp32, so normalization is a pure scale; a nonzero-min fallback
is folded in via two extra constant K-rows).

Device work per mask = one outer product = tiny matmuls on the tensor engine:
  - rows of the mask are interleaved 2-per-partition: psum[p, r*256+j] =
    mask[2p+r, j], so the [128,512] PSUM tile maps to a fully CONTIGUOUS
    256KB HBM range (partition p <-> bytes [p*2K,(p+1)*2K)).
  - two matmuls (r=0 even rows, r=1 odd rows), K=6, N=256, sharing one rhs.
  - operands are bf16 hi/lo error-compensated pairs: exf = eh + el,
    ey = yh + yl, product = eh*yh + el*yh + eh*yl + el*yl accumulated in
    fp32 PSUM -> ~8e-6 relative error at full bf16 PE speed.
  - PSUM -> SBUF copy alternates Vector/Scalar engines, then one 256KB
    contiguous DMA per mask.

Sharding: pure data parallel, batch 16 -> 8 cores x 2 samples.
"""

import math

import ml_dtypes
import numpy as np

import concourse.mybir as mybir
import concourse.tile as tile
from concourse import bacc
from concourse.bass_utils import run_bass_kernel_spmd

BF16 = ml_dtypes.bfloat16

B, N_MASK, H, W = 16, 50, 256, 256
SCALE_FACTOR = 50.0
N_CORES = 8
B_PER_CORE = B // N_CORES          # 2
M_PER_CORE = B_PER_CORE * N_MASK   # 100 masks per core
KROWS = 6                          # 4 product rows + 2 offset rows
GM = 10                            # masks per input-DMA group

# channel shuffle: out[:, c] = masks[:, PERM[c]]
PERM = np.arange(N_MASK).reshape(N_MASK // 2, 2).T.reshape(-1)

_NC_CACHE = {}
LAST_RESULTS = None


def _build_nc(
    g_out=1,          # masks per output DMA (1 = fully-contiguous per-mask DMAs)
    in_gpsimd=True,   # issue input DMAs from GPSIMD (SWDGE) instead of SP
    # NOTE: issuing output DMAs from the ACT sequencer (nc.scalar.dma_start)
    # crashes the exec unit on this runtime (NRT_EXEC_UNIT_UNRECOVERABLE),
    # and models identically to SP-only issue — keep alt_dma False.
    alt_dma=False,    # alternate output-DMA issue between SP and ACT sequencers
    out_bufs=8,
    psum_bufs=6,
    gm=GM,            # masks per input DMA
):
    """One-core program; run SPMD on 8 cores with different inputs."""
    assert M_PER_CORE % gm == 0 and gm % g_out == 0, (gm, g_out)
    nc = bacc.Bacc(
        "TRN2",
        target_bir_lowering=False,
        debug=False,
        num_devices=N_CORES,
    )
    inp = nc.dram_tensor(
        "packed", [KROWS, M_PER_CORE * 512], mybir.dt.bfloat16, kind="ExternalInput"
    )
    out = nc.dram_tensor(
        "masks", [M_PER_CORE, 128, 512], mybir.dt.float32, kind="ExternalOutput"
    )
    with tile.TileContext(nc) as tc:
        with (
            tc.tile_pool(name="inp", bufs=3) as in_pool,
            tc.tile_pool(name="outp", bufs=out_bufs) as out_pool,
            tc.tile_pool(name="psum", bufs=psum_bufs, space="PSUM") as psum_pool,
        ):
            n_groups = M_PER_CORE // gm
            for g in range(n_groups):
                it = in_pool.tile([KROWS, gm * 512], mybir.dt.bfloat16)
                in_eng = nc.gpsimd if in_gpsimd else nc.sync
                in_eng.dma_start(it[:], inp[:, g * gm * 512 : (g + 1) * gm * 512])
                for mo in range(gm // g_out):
                    ot = out_pool.tile([128, g_out, 512], mybir.dt.float32)
                    for mi in range(g_out):
                        m = g * gm + mo * g_out + mi
                        base = (mo * g_out + mi) * 512
                        lhsT0 = it[:, base : base + 128]
                        lhsT1 = it[:, base + 128 : base + 256]
                        rhs = it[:, base + 256 : base + 512]
                        ps = psum_pool.tile([128, 512], mybir.dt.float32)
                        # one accumulation group filling disjoint bank halves
                        nc.tensor.matmul(
                            ps[:, 0:256], lhsT0, rhs, start=True, stop=False
                        )
                        nc.tensor.matmul(
                            ps[:, 256:512], lhsT1, rhs, start=False, stop=True
                        )
                        if m % 2 == 0:
                            nc.vector.tensor_copy(ot[:, mi], ps[:])
                        else:
                            nc.scalar.copy(ot[:, mi], ps[:])
                    m0 = g * gm + mo * g_out
                    dst = out[m0 : m0 + g_out].rearrange("m p f -> p m f")
                    out_eng = nc.scalar if (alt_dma and mo % 2 == 1) else nc.sync
                    out_eng.dma_start(dst, ot[:])
    nc.compile()
    return nc


def _bf16_split(x):
    """x (f64) -> (hi, lo) bf16 with hi+lo ~= x to ~2^-18 relative."""
    hi = x.astype(BF16)
    lo = (x - hi.astype(np.float64)).astype(BF16)
    return hi, lo


def _host_factors(boxes):
    """Mimic the fp32 reference chain, then fold normalization.

    Returns exf [B,N,H] f64, ey [B,N,W] f64, d [B] f64 (offset, ==0 when the
    per-sample min underflows, which it always does for this regime).
    """
    boxes = np.asarray(boxes, np.float32)
    x, y, w, h = boxes[..., 0], boxes[..., 1], boxes[..., 2], boxes[..., 3]
    xc = x + np.float32(np.floor(w / np.float32(2.0)))
    yc = y + np.float32(np.floor(h / np.float32(2.0)))

    gx = np.round(np.linspace(np.float32(0.0), np.float32(H), H, dtype=np.float32))
    gy = np.round(np.linspace(np.float32(0.0), np.float32(W), W, dtype=np.float32))

    # fp32 arithmetic chain exactly like the jax reference
    dx = gx[None, None, :] - xc[..., None]
    dx2 = (dx * dx) / (np.float32(0.25) * w)[..., None]          # f32 [B,N,H]
    dy = gy[None, None, :] - yc[..., None]
    dy2 = (dy * dy) / (np.float32(0.25) * h)[..., None]          # f32 [B,N,W]

    ex = np.exp(np.float64(-0.5) * dx2.astype(np.float64))       # f64 [B,N,H]
    ey = np.exp(np.float64(-0.5) * dy2.astype(np.float64))       # f64 [B,N,W]

    det = (np.float32(0.0625) * w * h).astype(np.float64)        # [B,N]
    factor = (1.0 / (2.0 * math.pi)) * det ** -0.5               # f64 [B,N]

    m_max = factor * ex.max(-1) * ey.max(-1)                     # [B,N]
    m_min = factor * ex.min(-1) * ey.min(-1)
    # cast through f32 so fp32 underflow to 0 is reproduced
    mx = m_max.max(1).astype(np.float32).astype(np.float64)      # [B]
    mn = m_min.min(1).astype(np.float32).astype(np.float64)      # [B]

    a = SCALE_FACTOR / (mx - mn)                                 # [B]
    d = a * mn                                                   # [B]
    exf = a[:, None, None] * factor[..., None] * ex              # f64 [B,N,H]
    return exf, ey, d


def _pack_core_inputs(exf, ey, d):
    """Build the per-core packed [KROWS, M*512] bf16 operand arrays.

    Per mask slot m (= sample s * 50 + output channel c, mask n = PERM[c]):
      free [0:128)   lhsT for even rows r=0:  rows k: eh[0::2], el[0::2],
                     eh[0::2], el[0::2], 1, 1
      free [128:256) lhsT for odd rows r=1 (same with [1::2])
      free [256:512) rhs rows k: yh, yh, yl, yl, dh, dl  (dh+dl ~= -d)
    """
    eh, el = _bf16_split(exf)     # [B,N,H] bf16
    yh, yl = _bf16_split(ey)      # [B,N,W]
    dh, dl = _bf16_split(-d)      # [B]

    packed_all = []
    for core in range(N_CORES):
        pk = np.zeros((KROWS, M_PER_CORE, 512), dtype=BF16)
        for s in range(B_PER_CORE):
            b = core * B_PER_CORE + s
            sl = slice(s * N_MASK, (s + 1) * N_MASK)
            # [N,H] for this sample's masks in output-channel order
            ehb, elb = eh[b][PERM], el[b][PERM]
            yhb, ylb = yh[b][PERM], yl[b][PERM]
            lrows = (ehb, elb, ehb, elb)
            rrows = (yhb, yhb, ylb, ylb)
            for k in range(4):
                pk[k, sl, 0:128] = lrows[k][:, 0::2]
                pk[k, sl, 128:256] = lrows[k][:, 1::2]
                pk[k, sl, 256:512] = rrows[k]
            # offset rows: ones x (-d) split
            pk[4, sl, 0:256] = BF16(1.0)
            pk[5, sl, 0:256] = BF16(1.0)
            pk[4, sl, 256:512] = dh[b]
            pk[5, sl, 256:512] = dl[b]
        packed_all.append(np.ascontiguousarray(pk.reshape(KROWS, M_PER_CORE * 512)))
    return packed_all


def kernel(boxes, labels, fms_h, fms_w, trace=False, trace_cores=None):
    global LAST_RESULTS
    assert int(np.asarray(fms_h)) == H and int(np.asarray(fms_w)) == W

    labels = np.asarray(labels, np.float32)
    exf, ey, d = _host_factors(boxes)
    packed_all = _pack_core_inputs(exf, ey, d)

    if "nc" not in _NC_CACHE:
        import os

        cfg = {}
        if os.environ.get("KERNEL_CFG"):
            for kv in os.environ["KERNEL_CFG"].split(","):
                k, v = kv.split("=")
                cfg[k] = v.lower() == "true" if v.lower() in ("true", "false") else int(v)
        _NC_CACHE["nc"] = _build_nc(**cfg)
    nc = _NC_CACHE["nc"]

    in_maps = [{"packed": packed_all[c]} for c in range(N_CORES)]
    kwargs = {}
    if trace:
        kwargs["trace"] = True
        if trace_cores is not None:
            kwargs["trace_cores"] = trace_cores
    try:
        res = run_bass_kernel_spmd(nc, in_maps, core_ids=list(range(N_CORES)), **kwargs)
    except ModuleNotFoundError:
        if not trace:
            raise
        # NTFF profiling hook unavailable in this environment — run untraced.
        res = run_bass_kernel_spmd(nc, in_maps, core_ids=list(range(N_CORES)))
    LAST_RESULTS = res

    attention_masks = np.empty((B, N_MASK, H, W), np.float32)
    for core in range(N_CORES):
        arr = res.results[core]["masks"].reshape(B_PER_CORE, N_MASK, H, W)
        attention_masks[core * B_PER_CORE : (core + 1) * B_PER_CORE] = arr

    attention_labels = labels[:, :, 0][:, PERM].astype(np.float32)
    return attention_masks, attention_labels


# revision 17
# speedup vs baseline: 1.0021x; 1.0021x over previous
"""Trainium2 Bass kernel for nn_AttentionHead (Gaussian mask rasterization).

Reference computation (per batch sample b of 16, per mask n of 50):
    mask[n,i,j] = factor[n] * exp(-0.5*(dx2[n,i] + dy2[n,j]))     [256,256]
    out = (mask - min) / (max - min) * 50         (min/max over all n,i,j of b)
    channel-shuffled on n, labels likewise.

The Gaussian is separable: mask[i,j] = exf[i] * ey[j] with all the
normalization folded into exf on the host (the per-sample min underflows to
exactly 0 in fp32, so normalization is a pure scale; a nonzero-min fallback
is folded in via two extra constant K-rows).

Device work per mask = one outer product = tiny matmuls on the tensor engine:
  - rows of the mask are interleaved 2-per-partition: psum[p, r*256+j] =
    mask[2p+r, j], so the [128,512] PSUM tile maps to a fully CONTIGUOUS
    256KB HBM range (partition p <-> bytes [p*2K,(p+1)*2K)).
  - two matmuls (r=0 even rows, r=1 odd rows), K=6, N=256, sharing one rhs.
  - operands are bf16 hi/lo error-compensated pairs: exf = eh + el,
    ey = yh + yl, product = eh*yh + el*yh + eh*yl + el*yl accumulated in
    fp32 PSUM -> ~8e-6 relative error at full bf16 PE speed.
  - PSUM -> SBUF copy alternates Vector/Scalar engines, then one 256KB
    contiguous DMA per mask.

Sharding: pure data parallel, batch 16 -> 8 cores x 2 samples.
"""

import math

import ml_dtypes
import numpy as np

import concourse.mybir as mybir
import concourse.tile as tile
from concourse import bacc
from concourse.bass_utils import run_bass_kernel_spmd

BF16 = ml_dtypes.bfloat16

B, N_MASK, H, W = 16, 50, 256, 256
SCALE_FACTOR = 50.0
N_CORES = 8
B_PER_CORE = B // N_CORES          # 2
M_PER_CORE = B_PER_CORE * N_MASK   # 100 masks per core
KROWS = 6                          # 4 product rows + 2 offset rows
GM = 10                            # masks per input-DMA group

# channel shuffle: out[:, c] = masks[:, PERM[c]]
PERM = np.arange(N_MASK).reshape(N_MASK // 2, 2).T.reshape(-1)

_NC_CACHE = {}
LAST_RESULTS = None


def _build_nc(
    g_out=1,          # masks per output DMA (1 = fully-contiguous per-mask DMAs)
    in_gpsimd=True,   # issue input DMAs from GPSIMD (SWDGE) instead of SP
    # NOTE: issuing output DMAs from the ACT sequencer (nc.scalar.dma_start)
    # crashes the exec unit on this runtime (NRT_EXEC_UNIT_UNRECOVERABLE),
    # and models identically to SP-only issue — keep alt_dma False.
    alt_dma=False,    # alternate output-DMA issue between SP and ACT sequencers
    out_bufs=8,
    psum_bufs=6,
    gm=GM,            # masks per input DMA
    in_bufs=5,
    ramp=False,       # smaller leading input groups for faster pipeline start
):
    """One-core program; run SPMD on 8 cores with different inputs."""
    assert M_PER_CORE % gm == 0 and gm % g_out == 0, (gm, g_out)
    if ramp:
        group_sizes = [2, 4, 4] + [gm] * ((M_PER_CORE - 10) // gm)
        assert sum(group_sizes) == M_PER_CORE and g_out == 1
    else:
        group_sizes = [gm] * (M_PER_CORE // gm)
    nc = bacc.Bacc(
        "TRN2",
        target_bir_lowering=False,
        debug=False,
        num_devices=N_CORES,
    )
    inp = nc.dram_tensor(
        "packed", [KROWS, M_PER_CORE * 512], mybir.dt.bfloat16, kind="ExternalInput"
    )
    out = nc.dram_tensor(
        "masks", [M_PER_CORE, 128, 512], mybir.dt.float32, kind="ExternalOutput"
    )
    with tile.TileContext(nc) as tc:
        with (
            tc.tile_pool(name="inp", bufs=in_bufs) as in_pool,
            tc.tile_pool(name="outp", bufs=out_bufs) as out_pool,
            tc.tile_pool(name="psum", bufs=psum_bufs, space="PSUM") as psum_pool,
        ):
            g_start = 0
            for gs in group_sizes:
                it = in_pool.tile([KROWS, gm * 512], mybir.dt.bfloat16,
                                  tag="it")
                in_eng = nc.gpsimd if in_gpsimd else nc.sync
                in_eng.dma_start(
                    it[:, : gs * 512],
                    inp[:, g_start * 512 : (g_start + gs) * 512],
                )
                for mo in range(gs // g_out):
                    ot = out_pool.tile([128, g_out, 512], mybir.dt.float32)
                    for mi in range(g_out):
                        m = g_start + mo * g_out + mi
                        base = (mo * g_out + mi) * 512
                        lhsT0 = it[:, base : base + 128]
                        lhsT1 = it[:, base + 128 : base + 256]
                        rhs = it[:, base + 256 : base + 512]
                        ps = psum_pool.tile([128, 512], mybir.dt.float32)
                        # one accumulation group filling disjoint bank halves
                        nc.tensor.matmul(
                            ps[:, 0:256], lhsT0, rhs, start=True, stop=False
                        )
                        nc.tensor.matmul(
                            ps[:, 256:512], lhsT1, rhs, start=False, stop=True
                        )
                        if m % 2 == 0:
                            nc.vector.tensor_copy(ot[:, mi], ps[:])
                        else:
                            nc.scalar.copy(ot[:, mi], ps[:])
                    m0 = g_start + mo * g_out
                    dst = out[m0 : m0 + g_out].rearrange("m p f -> p m f")
                    out_eng = nc.scalar if (alt_dma and mo % 2 == 1) else nc.sync
                    out_eng.dma_start(dst, ot[:])
                g_start += gs
    nc.compile()
    return nc


def _bf16_split(x):
    """x (f64) -> (hi, lo) bf16 with hi+lo ~= x to ~2^-18 relative."""
    hi = x.astype(BF16)
    lo = (x - hi.astype(np.float64)).astype(BF16)
    return hi, lo


def _host_factors(boxes):
    """Mimic the fp32 reference chain, then fold normalization.

    Returns exf [B,N,H] f64, ey [B,N,W] f64, d [B] f64 (offset, ==0 when the
    per-sample min underflows, which it always does for this regime).
    """
    boxes = np.asarray(boxes, np.float32)
    x, y, w, h = boxes[..., 0], boxes[..., 1], boxes[..., 2], boxes[..., 3]
    xc = x + np.float32(np.floor(w / np.float32(2.0)))
    yc = y + np.float32(np.floor(h / np.float32(2.0)))

    gx = np.round(np.linspace(np.float32(0.0), np.float32(H), H, dtype=np.float32))
    gy = np.round(np.linspace(np.float32(0.0), np.float32(W), W, dtype=np.float32))

    # fp32 arithmetic chain exactly like the jax reference
    dx = gx[None, None, :] - xc[..., None]
    dx2 = (dx * dx) / (np.float32(0.25) * w)[..., None]          # f32 [B,N,H]
    dy = gy[None, None, :] - yc[..., None]
    dy2 = (dy * dy) / (np.float32(0.25) * h)[..., None]          # f32 [B,N,W]

    ex = np.exp(np.float64(-0.5) * dx2.astype(np.float64))       # f64 [B,N,H]
    ey = np.exp(np.float64(-0.5) * dy2.astype(np.float64))       # f64 [B,N,W]

    det = (np.float32(0.0625) * w * h).astype(np.float64)        # [B,N]
    factor = (1.0 / (2.0 * math.pi)) * det ** -0.5               # f64 [B,N]

    m_max = factor * ex.max(-1) * ey.max(-1)                     # [B,N]
    m_min = factor * ex.min(-1) * ey.min(-1)
    # cast through f32 so fp32 underflow to 0 is reproduced
    mx = m_max.max(1).astype(np.float32).astype(np.float64)      # [B]
    mn = m_min.min(1).astype(np.float32).astype(np.float64)      # [B]

    a = SCALE_FACTOR / (mx - mn)                                 # [B]
    d = a * mn                                                   # [B]
    exf = a[:, None, None] * factor[..., None] * ex              # f64 [B,N,H]
    return exf, ey, d


def _pack_core_inputs(exf, ey, d):
    """Build the per-core packed [KROWS, M*512] bf16 operand arrays.

    Per mask slot m (= sample s * 50 + output channel c, mask n = PERM[c]):
      free [0:128)   lhsT for even rows r=0:  rows k: eh[0::2], el[0::2],
                     eh[0::2], el[0::2], 1, 1
      free [128:256) lhsT for odd rows r=1 (same with [1::2])
      free [256:512) rhs rows k: yh, yh, yl, yl, dh, dl  (dh+dl ~= -d)
    """
    eh, el = _bf16_split(exf)     # [B,N,H] bf16
    yh, yl = _bf16_split(ey)      # [B,N,W]
    dh, dl = _bf16_split(-d)      # [B]

    packed_all = []
    for core in range(N_CORES):
        pk = np.zeros((KROWS, M_PER_CORE, 512), dtype=BF16)
        for s in range(B_PER_CORE):
            b = core * B_PER_CORE + s
            sl = slice(s * N_MASK, (s + 1) * N_MASK)
            # [N,H] for this sample's masks in output-channel order
            ehb, elb = eh[b][PERM], el[b][PERM]
            yhb, ylb = yh[b][PERM], yl[b][PERM]
            lrows = (ehb, elb, ehb, elb)
            rrows = (yhb, yhb, ylb, ylb)
            for k in range(4):
                pk[k, sl, 0:128] = lrows[k][:, 0::2]
                pk[k, sl, 128:256] = lrows[k][:, 1::2]
                pk[k, sl, 256:512] = rrows[k]
            # offset rows: ones x (-d) split
            pk[4, sl, 0:256] = BF16(1.0)
            pk[5, sl, 0:256] = BF16(1.0)
            pk[4, sl, 256:512] = dh[b]
            pk[5, sl, 256:512] = dl[b]
        packed_all.append(np.ascontiguousarray(pk.reshape(KROWS, M_PER_CORE * 512)))
    return packed_all


def kernel(boxes, labels, fms_h, fms_w, trace=False, trace_cores=None):
    global LAST_RESULTS
    assert int(np.asarray(fms_h)) == H and int(np.asarray(fms_w)) == W

    labels = np.asarray(labels, np.float32)
    exf, ey, d = _host_factors(boxes)
    packed_all = _pack_core_inputs(exf, ey, d)

    if "nc" not in _NC_CACHE:
        import os

        cfg = {}
        if os.environ.get("KERNEL_CFG"):
            for kv in os.environ["KERNEL_CFG"].split(","):
                k, v = kv.split("=")
                cfg[k] = v.lower() == "true" if v.lower() in ("true", "false") else int(v)
        _NC_CACHE["nc"] = _build_nc(**cfg)
    nc = _NC_CACHE["nc"]

    in_maps = [{"packed": packed_all[c]} for c in range(N_CORES)]
    kwargs = {}
    if trace:
        kwargs["trace"] = True
        if trace_cores is not None:
            kwargs["trace_cores"] = trace_cores
    try:
        res = run_bass_kernel_spmd(nc, in_maps, core_ids=list(range(N_CORES)), **kwargs)
    except ModuleNotFoundError:
        if not trace:
            raise
        # NTFF profiling hook unavailable in this environment — run untraced.
        res = run_bass_kernel_spmd(nc, in_maps, core_ids=list(range(N_CORES)))
    LAST_RESULTS = res

    attention_masks = np.empty((B, N_MASK, H, W), np.float32)
    for core in range(N_CORES):
        arr = res.results[core]["masks"].reshape(B_PER_CORE, N_MASK, H, W)
        attention_masks[core * B_PER_CORE : (core + 1) * B_PER_CORE] = arr

    attention_labels = labels[:, :, 0][:, PERM].astype(np.float32)
    return attention_masks, attention_labels
